# revision 22
# baseline (speedup 1.0000x reference)
"""LoMoE output head kernel for 8 Trainium2 NeuronCores.

Strategy (tensor-parallel over in_features):
  - The dominant cost is streaming x [32,21,512,64] (88 MB f32) and the
    projection weights through HBM.  We shard the 32768-long feature axis
    8 ways: core k owns features [4096k, 4096(k+1)).
  - Each core computes a partial [160, 672] = [W_base | lora_A_flat] @ x_shard.T
    in bf16 (f32 PSUM accumulation).  That one fused matmul covers both the
    base projection (96 rows) and all-expert LoRA "temp" (64 = 8 experts x
    rank 8 rows).
  - Host gathers the 8 partials, sums them (the TP all-reduce), applies the
    router (top-2 softmax gating, computed on host from the pooled means),
    the rank-8 expert combine, and the bias.
"""

import numpy as np
import ml_dtypes


def _install_ntff_hook():
    """concourse's trace path (enabled by BASS_TRACE=1 in the environment)
    imports antenv.axon_hooks, which this container image lacks.  Register a
    shim (and the ctypes-based NTFF hook when available) so tracing works
    instead of crashing."""
    import sys
    import types

    if "antenv.axon_hooks" in sys.modules:
        return
    try:
        import antenv
    except ImportError:
        return
    hooks = types.ModuleType("antenv.axon_hooks")
    state = [None]
    hooks.set_axon_ntff_profile_hook = lambda h: state.__setitem__(0, h)
    hooks.get_axon_ntff_profile_hook = lambda: state[0]
    sys.modules["antenv.axon_hooks"] = hooks
    antenv.axon_hooks = hooks
    try:
        from trn_agent_boot.trn_boot import _ntff_profile_via_ctypes

        hooks.set_axon_ntff_profile_hook(
            _ntff_profile_via_ctypes("/opt/axon/libaxon_pjrt.so")
        )
    except Exception:
        pass


_install_ntff_hook()

import concourse.bass as bass
import concourse.mybir as mybir
import concourse.tile as tile
from concourse import bacc
from concourse.bass_utils import run_bass_kernel_spmd

B, V, D, P = 32, 21, 512, 64
T = B * V                  # 672 tokens
IN = D * P                 # 32768
OUT = 96
E, RK = 8, 8
M2 = E * RK                # 64 lora rows
MTOT = OUT + M2            # 160
NC = 8                     # cores
F = IN // NC               # 4096 features per core
CH = F // 128              # 32 K-chunks of 128
TOP_K = 2
SCALING = 16.0 / 8.0

# x/w are DMA'd in progressive chunk-groups: small first groups so the PE
# starts early, large later groups for DMA efficiency.
GRPS = [1, 1] + [2] * 15
assert sum(GRPS) == CH
TW = T + MTOT              # tokens + weight cols packed per chunk row
NT = 2                     # token tiles (PSUM bank holds <=512 f32)
TT = T // NT               # 336

BF16 = mybir.dt.bfloat16
F32 = mybir.dt.float32
np_bf16 = ml_dtypes.bfloat16


def _build_bass():
    nc = bacc.Bacc("TRN2", target_bir_lowering=False, debug=False)
    # xw_sb[p, c, 0:T]      = x_flat[t, shard_base + c*128 + p]   (bf16)
    # xw_sb[p, c, T:T+MTOT] = Wcat[m, shard_base + c*128 + p]     (bf16)
    xw_d = nc.dram_tensor("xw_sb", [128, CH, TW], BF16, kind="ExternalInput")
    # rows 0:96 base, 96:160 lora partial (even chunks), 160:224 lora partial
    # (odd chunks) -- host adds the two lora partials.
    o_d = nc.dram_tensor("out_part", [OUT + 2 * M2, T], F32, kind="ExternalOutput")

    with tile.TileContext(nc) as tc:
        with (
            tc.tile_pool(name="wp", bufs=1) as wp,
            tc.tile_pool(name="xp", bufs=1) as xp,
            tc.tile_pool(name="pp", bufs=1, space="PSUM") as pp,
            tc.tile_pool(name="op", bufs=1) as op,
        ):
            xw_tiles = []
            c0 = 0
            for g, cpg in enumerate(GRPS):
                xwt = xp.tile([128, cpg, TW], BF16, name=f"xw{g}", tag=f"xw{g}")
                eng = nc.sync if g % 2 == 0 else nc.scalar
                eng.dma_start(xwt[:], xw_d[:, c0:c0 + cpg, :])
                xw_tiles.append(xwt)
                c0 += cpg

            # base accumulators: [96, TT] x2; lora accumulators: even chunks
            # at PSUM partitions 0:64, odd chunks at partitions 64:128 (col
            # tiling lets the even/odd lora matmuls of a chunk pair stream
            # concurrently through disjoint column groups of the PE array).
            psB = [pp.tile([OUT, TT], F32, name=f"psB{j}", tag=f"psB{j}") for j in range(NT)]
            psL = [pp.tile([128, TT], F32, name=f"psL{j}", tag=f"psL{j}") for j in range(NT)]

            def chunk_tiles(c):
                cl = c
                for g, cpg in enumerate(GRPS):
                    if cl < cpg:
                        return xw_tiles[g], cl
                    cl -= cpg
                raise AssertionError

            # process 4 chunks per block: 4 base passes then 2 packed lora
            # passes -- fewer base<->lora stationary transitions per chunk.
            BLK = 4
            for blk in range(CH // BLK):
                cs = [blk * BLK + i for i in range(BLK)]
                first = blk == 0
                last = blk == CH // BLK - 1
                for idx, c in enumerate(cs):
                    t_, cl_ = chunk_tiles(c)
                    for j in range(NT):
                        nc.tensor.matmul(
                            psB[j][:],
                            t_[:, cl_, T:T + OUT],
                            t_[:, cl_, j * TT:(j + 1) * TT],
                            start=first and idx == 0,
                            stop=last and idx == BLK - 1,
                        )
                # packed lora passes: even chunk -> cols/partitions 0:64,
                # odd chunk -> cols/partitions 64:128, concurrent streams.
                for pi in range(BLK // 2):
                    ce, co = cs[2 * pi], cs[2 * pi + 1]
                    te, cle = chunk_tiles(ce)
                    to, clo = chunk_tiles(co)
                    for j in range(NT):
                        nc.tensor.matmul(
                            psL[j][0:M2, :],
                            te[:, cle, T + OUT:T + MTOT],
                            te[:, cle, j * TT:(j + 1) * TT],
                            start=first and pi == 0,
                            stop=last and pi == BLK // 2 - 1,
                        )
                        nc.tensor.matmul(
                            psL[j][M2:128, :],
                            to[:, clo, T + OUT:T + MTOT],
                            to[:, clo, j * TT:(j + 1) * TT],
                            start=first and pi == 0,
                            stop=last and pi == BLK // 2 - 1,
                        )

            # separate SBUF tiles per token half so each store can launch as
            # soon as its own copy lands; spread across both HWDGE rings.
            for j in range(NT):
                obj = op.tile([OUT, TT], F32, name=f"ob{j}", tag=f"ob{j}")
                olj = op.tile([128, TT], F32, name=f"ol{j}", tag=f"ol{j}")
                nc.vector.tensor_copy(obj[:], psB[j][:])
                nc.vector.tensor_copy(olj[:], psL[j][:])
                eng_b = nc.scalar if j == 0 else nc.sync
                eng_l = nc.sync if j == 0 else nc.scalar
                eng_b.dma_start(o_d[0:OUT, j * TT:(j + 1) * TT], obj[:])
                eng_l.dma_start(o_d[OUT:OUT + 2 * M2, j * TT:(j + 1) * TT], olj[:])

    nc.compile()
    return nc


def _host_router(x, W1, b1, W2, b2):
    """Top-2 softmax gating, computed exactly (f64) on the pooled means."""
    pooled = x.astype(np.float64).mean(axis=(1, 3))            # [B, D]
    h = np.maximum(pooled @ W1.astype(np.float64).T + b1, 0.0)
    logits = h @ W2.astype(np.float64).T + b2
    z = np.exp(logits - logits.max(-1, keepdims=True))
    probs = z / z.sum(-1, keepdims=True)
    topi = np.argsort(-probs, axis=-1, kind="stable")[:, :TOP_K]
    topw = np.take_along_axis(probs, topi, axis=-1)
    topw = topw / np.clip(topw.sum(-1, keepdims=True), 1e-6, None)
    w_full = np.zeros((B, E))
    np.put_along_axis(w_full, topi, topw, axis=-1)
    return probs.astype(np.float32), w_full


def _run(inputs, trace=False, repeats=1):
    x = np.asarray(inputs["x"], dtype=np.float32)
    W_base = np.asarray(inputs["W_base"], dtype=np.float32)
    b_base = np.asarray(inputs["b_base"], dtype=np.float32)
    lora_A = np.asarray(inputs["lora_A"], dtype=np.float32)
    lora_B = np.asarray(inputs["lora_B"], dtype=np.float32)

    flat = x.reshape(T, IN)
    probs, w_full = _host_router(
        x,
        np.asarray(inputs["W1"], dtype=np.float32),
        np.asarray(inputs["b1"], dtype=np.float32),
        np.asarray(inputs["W2"], dtype=np.float32),
        np.asarray(inputs["b2"], dtype=np.float32),
    )

    Wcat = np.concatenate([W_base, lora_A.reshape(M2, IN)], axis=0)  # [160, IN]

    in_maps = []
    for k in range(NC):
        sl = slice(k * F, (k + 1) * F)
        # [F, T] -> [CH, 128, T] -> [128, CH, T], plus W cols appended
        xsb = flat[:, sl].T.reshape(CH, 128, T).transpose(1, 0, 2)
        wsb = Wcat[:, sl].T.reshape(CH, 128, MTOT).transpose(1, 0, 2)
        xw = np.concatenate([xsb, wsb], axis=2).astype(np_bf16)
        in_maps.append({"xw_sb": np.ascontiguousarray(xw)})

    nc = _build_bass()
    res = run_bass_kernel_spmd(nc, in_maps, core_ids=list(range(NC)), trace=trace)
    if repeats > 1:
        times = [res.exec_time_ns]
        for _ in range(repeats - 1):
            r2 = run_bass_kernel_spmd(nc, in_maps, core_ids=list(range(NC)), trace=trace)
            times.append(r2.exec_time_ns)
            if r2.exec_time_ns is not None and (
                res.exec_time_ns is None or r2.exec_time_ns < res.exec_time_ns
            ):
                res = r2
        res.all_exec_times_ns = times

    total = np.zeros((OUT + 2 * M2, T), np.float32)
    for r in res.results:
        total += r["out_part"]

    base = total[:OUT].T + b_base                               # [T, 96]
    temp = total[OUT:OUT + M2] + total[OUT + M2:]               # [64, T]
    # Mb[b] @ temp[:, tokens of b] folds lora_B, gate weight and SCALING.
    Mb = SCALING * (w_full[:, :, None, None] * lora_B[None])    # [B, E, 96, R]
    Mb = np.transpose(Mb, (0, 2, 1, 3)).reshape(B, OUT, M2).astype(np.float32)
    out = np.empty((B, V, OUT), np.float32)
    for b in range(B):
        tb = temp[:, b * V:(b + 1) * V]
        out[b] = base[b * V:(b + 1) * V] + (Mb[b] @ tb).T
    return out, probs, res


def kernel(**inputs):
    out, probs, _ = _run(inputs)
    return out, probs


# revision 23
# speedup vs baseline: 1.0331x; 1.0331x over previous
"""LoMoE output head kernel for 8 Trainium2 NeuronCores.

Strategy (tensor-parallel over in_features):
  - The dominant cost is streaming x [32,21,512,64] (88 MB f32) and the
    projection weights through HBM.  We shard the 32768-long feature axis
    8 ways: core k owns features [4096k, 4096(k+1)).
  - Each core computes a partial [160, 672] = [W_base | lora_A_flat] @ x_shard.T
    in bf16 (f32 PSUM accumulation).  That one fused matmul covers both the
    base projection (96 rows) and all-expert LoRA "temp" (64 = 8 experts x
    rank 8 rows).
  - Host gathers the 8 partials, sums them (the TP all-reduce), applies the
    router (top-2 softmax gating, computed on host from the pooled means),
    the rank-8 expert combine, and the bias.
"""

import numpy as np
import ml_dtypes


def _install_ntff_hook():
    """concourse's trace path (enabled by BASS_TRACE=1 in the environment)
    imports antenv.axon_hooks, which this container image lacks.  Register a
    shim (and the ctypes-based NTFF hook when available) so tracing works
    instead of crashing."""
    import sys
    import types

    if "antenv.axon_hooks" in sys.modules:
        return
    try:
        import antenv
    except ImportError:
        return
    hooks = types.ModuleType("antenv.axon_hooks")
    state = [None]
    hooks.set_axon_ntff_profile_hook = lambda h: state.__setitem__(0, h)
    hooks.get_axon_ntff_profile_hook = lambda: state[0]
    sys.modules["antenv.axon_hooks"] = hooks
    antenv.axon_hooks = hooks
    try:
        from trn_agent_boot.trn_boot import _ntff_profile_via_ctypes

        hooks.set_axon_ntff_profile_hook(
            _ntff_profile_via_ctypes("/opt/axon/libaxon_pjrt.so")
        )
    except Exception:
        pass


_install_ntff_hook()

import concourse.bass as bass
import concourse.mybir as mybir
import concourse.tile as tile
from concourse import bacc
from concourse.bass_utils import run_bass_kernel_spmd

B, V, D, P = 32, 21, 512, 64
T = B * V                  # 672 tokens
IN = D * P                 # 32768
OUT = 96
E, RK = 8, 8
M2 = E * RK                # 64 lora rows
MTOT = OUT + M2            # 160
NC = 8                     # cores
F = IN // NC               # 4096 features per core
CH = F // 128              # 32 K-chunks of 128
TOP_K = 2
SCALING = 16.0 / 8.0

# x/w are DMA'd in progressive chunk-groups: small first groups so the PE
# starts early, large later groups for DMA efficiency.
GRPS = [1, 1] + [2] * 14 + [1, 1]
assert sum(GRPS) == CH
TW = T + MTOT              # tokens + weight cols packed per chunk row
NT = 2                     # token tiles (PSUM bank holds <=512 f32)
TT = T // NT               # 336

BF16 = mybir.dt.bfloat16
F32 = mybir.dt.float32
np_bf16 = ml_dtypes.bfloat16


def _build_bass():
    nc = bacc.Bacc("TRN2", target_bir_lowering=False, debug=False)
    # xw_sb[p, c, 0:T]      = x_flat[t, shard_base + c*128 + p]   (bf16)
    # xw_sb[p, c, T:T+MTOT] = Wcat[m, shard_base + c*128 + p]     (bf16)
    xw_d = nc.dram_tensor("xw_sb", [128, CH, TW], BF16, kind="ExternalInput")
    # rows 0:96 base, 96:160 lora partial (even chunks), 160:224 lora partial
    # (odd chunks) -- host adds the two lora partials.
    o_d = nc.dram_tensor("out_part", [OUT + 2 * M2, T], BF16, kind="ExternalOutput")

    with tile.TileContext(nc) as tc:
        with (
            tc.tile_pool(name="wp", bufs=1) as wp,
            tc.tile_pool(name="xp", bufs=1) as xp,
            tc.tile_pool(name="pp", bufs=1, space="PSUM") as pp,
            tc.tile_pool(name="op", bufs=1) as op,
        ):
            xw_tiles = []
            c0 = 0
            for g, cpg in enumerate(GRPS):
                xwt = xp.tile([128, cpg, TW], BF16, name=f"xw{g}", tag=f"xw{g}")
                eng = nc.sync if g % 2 == 0 else nc.scalar
                eng.dma_start(xwt[:], xw_d[:, c0:c0 + cpg, :])
                xw_tiles.append(xwt)
                c0 += cpg

            # base accumulators: [96, TT] x2; lora accumulators: even chunks
            # at PSUM partitions 0:64, odd chunks at partitions 64:128 (col
            # tiling lets the even/odd lora matmuls of a chunk pair stream
            # concurrently through disjoint column groups of the PE array).
            psB = [pp.tile([OUT, TT], F32, name=f"psB{j}", tag=f"psB{j}") for j in range(NT)]
            psL = [pp.tile([128, TT], F32, name=f"psL{j}", tag=f"psL{j}") for j in range(NT)]

            def chunk_tiles(c):
                cl = c
                for g, cpg in enumerate(GRPS):
                    if cl < cpg:
                        return xw_tiles[g], cl
                    cl -= cpg
                raise AssertionError

            # process 4 chunks per block: 4 base passes then 2 packed lora
            # passes -- fewer base<->lora stationary transitions per chunk.
            BLK = 4

            def base_mm(c, start, stop):
                t_, cl_ = chunk_tiles(c)
                for j in range(NT):
                    nc.tensor.matmul(
                        psB[j][:],
                        t_[:, cl_, T:T + OUT],
                        t_[:, cl_, j * TT:(j + 1) * TT],
                        start=start,
                        stop=stop,
                    )

            def lora_pass(ce, co, start, stop):
                te, cle = chunk_tiles(ce)
                to, clo = chunk_tiles(co)
                for j in range(NT):
                    nc.tensor.matmul(
                        psL[j][0:M2, :],
                        te[:, cle, T + OUT:T + MTOT],
                        te[:, cle, j * TT:(j + 1) * TT],
                        start=start,
                        stop=stop,
                    )
                    nc.tensor.matmul(
                        psL[j][M2:128, :],
                        to[:, clo, T + OUT:T + MTOT],
                        to[:, clo, j * TT:(j + 1) * TT],
                        start=start,
                        stop=stop,
                    )

            NBLK = CH // BLK
            for blk in range(NBLK):
                cs = [blk * BLK + i for i in range(BLK)]
                first = blk == 0
                last = blk == NBLK - 1
                if not last:
                    for idx, c in enumerate(cs):
                        base_mm(c, first and idx == 0, False)
                    for pi in range(BLK // 2):
                        lora_pass(cs[2 * pi], cs[2 * pi + 1], first and pi == 0, False)
                else:
                    # pair-interleaved: the first pair's lora pass fills the
                    # PE wait for the final chunks' DMA.
                    base_mm(cs[0], False, False)
                    base_mm(cs[1], False, False)
                    lora_pass(cs[0], cs[1], False, False)
                    base_mm(cs[2], False, False)
                    base_mm(cs[3], False, True)
                    lora_pass(cs[2], cs[3], False, True)

            # separate SBUF tiles per token half so each store can launch as
            # soon as its own copy lands; spread across both HWDGE rings.
            for j in range(NT):
                obj = op.tile([OUT, TT], BF16, name=f"ob{j}", tag=f"ob{j}")
                olj = op.tile([128, TT], BF16, name=f"ol{j}", tag=f"ol{j}")
                nc.vector.tensor_copy(obj[:], psB[j][:])
                nc.vector.tensor_copy(olj[:], psL[j][:])
                eng_b = nc.scalar if j == 0 else nc.sync
                eng_l = nc.sync if j == 0 else nc.scalar
                eng_b.dma_start(o_d[0:OUT, j * TT:(j + 1) * TT], obj[:])
                eng_l.dma_start(o_d[OUT:OUT + 2 * M2, j * TT:(j + 1) * TT], olj[:])

    nc.compile()
    return nc


def _host_router(x, W1, b1, W2, b2):
    """Top-2 softmax gating, computed exactly (f64) on the pooled means."""
    pooled = x.astype(np.float64).mean(axis=(1, 3))            # [B, D]
    h = np.maximum(pooled @ W1.astype(np.float64).T + b1, 0.0)
    logits = h @ W2.astype(np.float64).T + b2
    z = np.exp(logits - logits.max(-1, keepdims=True))
    probs = z / z.sum(-1, keepdims=True)
    topi = np.argsort(-probs, axis=-1, kind="stable")[:, :TOP_K]
    topw = np.take_along_axis(probs, topi, axis=-1)
    topw = topw / np.clip(topw.sum(-1, keepdims=True), 1e-6, None)
    w_full = np.zeros((B, E))
    np.put_along_axis(w_full, topi, topw, axis=-1)
    return probs.astype(np.float32), w_full


def _run(inputs, trace=False, repeats=1):
    x = np.asarray(inputs["x"], dtype=np.float32)
    W_base = np.asarray(inputs["W_base"], dtype=np.float32)
    b_base = np.asarray(inputs["b_base"], dtype=np.float32)
    lora_A = np.asarray(inputs["lora_A"], dtype=np.float32)
    lora_B = np.asarray(inputs["lora_B"], dtype=np.float32)

    flat = x.reshape(T, IN)
    probs, w_full = _host_router(
        x,
        np.asarray(inputs["W1"], dtype=np.float32),
        np.asarray(inputs["b1"], dtype=np.float32),
        np.asarray(inputs["W2"], dtype=np.float32),
        np.asarray(inputs["b2"], dtype=np.float32),
    )

    Wcat = np.concatenate([W_base, lora_A.reshape(M2, IN)], axis=0)  # [160, IN]

    in_maps = []
    for k in range(NC):
        sl = slice(k * F, (k + 1) * F)
        # [F, T] -> [CH, 128, T] -> [128, CH, T], plus W cols appended
        xsb = flat[:, sl].T.reshape(CH, 128, T).transpose(1, 0, 2)
        wsb = Wcat[:, sl].T.reshape(CH, 128, MTOT).transpose(1, 0, 2)
        xw = np.concatenate([xsb, wsb], axis=2).astype(np_bf16)
        in_maps.append({"xw_sb": np.ascontiguousarray(xw)})

    nc = _build_bass()
    res = run_bass_kernel_spmd(nc, in_maps, core_ids=list(range(NC)), trace=trace)
    if repeats > 1:
        times = [res.exec_time_ns]
        for _ in range(repeats - 1):
            r2 = run_bass_kernel_spmd(nc, in_maps, core_ids=list(range(NC)), trace=trace)
            times.append(r2.exec_time_ns)
            if r2.exec_time_ns is not None and (
                res.exec_time_ns is None or r2.exec_time_ns < res.exec_time_ns
            ):
                res = r2
        res.all_exec_times_ns = times

    total = np.zeros((OUT + 2 * M2, T), np.float32)
    for r in res.results:
        total += r["out_part"].astype(np.float32)

    base = total[:OUT].T + b_base                               # [T, 96]
    temp = total[OUT:OUT + M2] + total[OUT + M2:]               # [64, T]
    # Mb[b] @ temp[:, tokens of b] folds lora_B, gate weight and SCALING.
    Mb = SCALING * (w_full[:, :, None, None] * lora_B[None])    # [B, E, 96, R]
    Mb = np.transpose(Mb, (0, 2, 1, 3)).reshape(B, OUT, M2).astype(np.float32)
    out = np.empty((B, V, OUT), np.float32)
    for b in range(B):
        tb = temp[:, b * V:(b + 1) * V]
        out[b] = base[b * V:(b + 1) * V] + (Mb[b] @ tb).T
    return out, probs, res


def kernel(**inputs):
    out, probs, _ = _run(inputs)
    return out, probs
